# revision 81
# baseline (speedup 1.0000x reference)
"""AngleEnsemble TRN2 kernel: von Mises mean-shift via Jacobi-Anger moments.

Math: softmax mixture w = (1/3) sum_m softmax(logits_m). Mean-shift iterates
theta <- atan2(S(theta), C(theta)) with C,S = sum_n w_n exp(kappa cos(theta-theta_n)) {cos,sin}theta_n.
Expanding exp(kappa cos phi) = I0 + 2 sum_k Ik cos(k phi) (truncated at K=8),
C and S become trig polynomials in theta whose per-batch coefficients are
linear in w: one fp16 matmul exp(logits) @ F' [360, NCOL] produces
[Z | coeffC | coeffS] (Z = softmax normalizer via the ones column; a global
1/64 scale keeps fp16 in range and cancels in the final normalize).

Schedule (per core, 8192 batch rows as 64 j-columns of 128):
- phase 1: 4 superchunks x 3 mixture heads; DMA (SP queue) -> exp (ACT,
  the 61us serial spine) -> 16x3 moment matmuls (PE, PSUM) -> merged 1/Z
  scale (ONE DVE scalar_tensor_tensor per (s,m) through a flattened
  [128,36,16] view: TensorScalarPtr ops allow only 2 free dims).
- phase 2: three 10-iteration mean-shift chains on DVE over uneven
  superchunks [12, 20, 20+12]: the small first chunk lets the first chain
  start early in the exp stream, and by the time the second chain ends
  both late coefficient sets are ready, so s2+s3 merge into one w32 chain
  that pays the per-op overhead once. Chains pipeline via fill closures
  (accumulation + head-post ops interleave into iteration stall points).
  Iteration = 12-op harmonic doubling (paired [128,2,ww,w] products), one
  merged C/S product, a 5-op fp16 add-tree reduce (cheaper than
  tensor_reduce: the packed adds get the DVE 2x mode), and a 5-op
  magic-rsqrt normalize (no Newton step; the scale error it leaves is a
  common factor per harmonic order and stays inside tolerance).
  gpsimd cannot run chains: TensorScalar/TensorScalarPtr-class ops and
  int32 shifts are not in its ISA, and cross-engine chain deps resolve
  against the tile scheduler's optimistic Pool timing.
- head MLP per chain (w=16): PE-transpose the final unit vectors to
  j-major, ACT-copy out of PSUM, DMA into the 5-row b-major fused input,
  5x128 + 128x2 PE matmuls with relu blocks alternating between ACT and
  DVE (pipelines the tail head), magic-rsqrt row normalize on DVE; the
  tail head's normalize+store is split in two halves pipelined against
  its own matmuls. Heads run in chain-completion order; their sync-queue
  DMAs stay monotone in readiness so the FIFO never head-of-line blocks.
  Output stores ride the ACT queue to keep the sync queue clear.
"""
import numpy as np
from contextlib import ExitStack

import concourse.bass as bass
import concourse.bacc as bacc
import concourse.mybir as mybir
from concourse.tile import TileContext
from concourse.bass_utils import run_bass_kernel_spmd

F32 = mybir.dt.float32
F16 = mybir.dt.float16
I32 = mybir.dt.int32
AF = mybir.ActivationFunctionType
OP = mybir.AluOpType
AX = mybir.AxisListType

M, B, N = 3, 65536, 360
NCORES = 8
BS = B // NCORES          # 8192 batch rows per core
KORD = 8                  # Jacobi-Anger truncation order
NPC = 2 * KORD + 1        # 17 real rows [1 | c_1..c_K | s_1..s_K]
PBLK = NPC + 1            # 18 = padded block (zero pad row)
NCOL = 1 + 2 * PBLK       # 37 = Z | coeffC+pad | coeffS+pad
NK = 120                  # n-chunk (3 chunks of 120 = 360)
SUP = 2048                # b superchunk for DMA/exp staging
NSUP = BS // SUP          # 4
JS = SUP // 128           # 16 j-columns per superchunk
NJ = BS // 128            # 64 column-groups of 128 b
MS_ITERS = 10
MAGIC = 0x5F3759DF
CI, SI = 1, 1 + KORD      # P row of c_1 / s_1
W1R = 5                   # fused rows: cos, sin, sv0, sv1, ones
SUPS = [12, 20, 20, 12]    # j-columns per superchunk (uneven: small s0 lets
                          # the first chain start ~11us earlier; exp total
                          # is unchanged)
SOFF = [0, 12, 32, 52]     # prefix offsets
NEWTON = False            # Newton step after magic rsqrt in chain normalize


def build(nc: bass.Bass):
    lg = nc.declare_dram_parameter("logitsT", [M, N, BS], F16, isOutput=False)
    sv = nc.declare_dram_parameter("sin_vecT", [3, BS], F16, isOutput=False)
    fp = nc.declare_dram_parameter("Fp", [3, NK, NCOL], F16, isOutput=False)
    w1b = nc.declare_dram_parameter("W1b", [W1R, 128], F16, isOutput=False)
    w2 = nc.declare_dram_parameter("W2", [128, 2], F16, isOutput=False)
    b2r = nc.declare_dram_parameter("b2r", [128, 2], F32, isOutput=False)
    eye = nc.declare_dram_parameter("eye", [128, 128], F16, isOutput=False)
    out = nc.declare_dram_parameter("out", [BS, 2], F32, isOutput=True)

    with TileContext(nc) as tc, ExitStack() as ctx:
        consts = ctx.enter_context(tc.tile_pool(name="consts", bufs=1))
        state = ctx.enter_context(tc.tile_pool(name="state", bufs=1))
        epool = ctx.enter_context(tc.tile_pool(name="epool", bufs=3))
        xpool = ctx.enter_context(tc.tile_pool(name="xpool", bufs=3))
        rpool = ctx.enter_context(tc.tile_pool(name="rpool", bufs=3))
        headp = ctx.enter_context(tc.tile_pool(name="headp", bufs=2))
        psum = ctx.enter_context(tc.tile_pool(name="psum", bufs=2, space="PSUM"))
        psumh = ctx.enter_context(tc.tile_pool(name="psumh", bufs=2, space="PSUM"))

        # ---- ACT table prime: pull the Exp table load off the critical path
        prime = consts.tile([1, 2], F16)
        nc.vector.memset(prime[:], 0.0)
        nc.scalar.activation(out=prime[:, 0:1], in_=prime[:, 1:2], func=AF.Exp)

        # ---- constants on the gpsimd (SWDGE) queue: keeps the SP queue
        # free so the first logit loads issue immediately ----
        fp_t = consts.tile([NK, 3, NCOL], F16)
        for k in range(3):
            nc.gpsimd.dma_start(out=fp_t[:, k, :], in_=fp[k])
        w1b_t = consts.tile([W1R, 128], F16)
        w2_t = consts.tile([128, 2], F16)
        b2r_t = consts.tile([128, 2], F32)
        eye_t = consts.tile([128, 128], F16)
        fusedX = state.tile([W1R, BS], F16)
        # int constants for the Pool-engine magic-rsqrt (gpsimd has no
        # TensorScalar-class ops, so shifts/offsets need tensor operands)
        ones_i = consts.tile([128, JS], I32)
        magic_i = consts.tile([128, JS], I32)
        nc.vector.memset(ones_i[:], 1)
        nc.vector.memset(magic_i[:], MAGIC)

        def load_late_consts():
            nc.gpsimd.dma_start(out=w1b_t[:], in_=w1b[:, :])
            nc.gpsimd.dma_start(out=w2_t[:], in_=w2[:, :])
            nc.gpsimd.dma_start(out=b2r_t[:], in_=b2r[:, :])
            nc.gpsimd.dma_start(out=eye_t[:], in_=eye[:, :])
            nc.gpsimd.dma_start(out=fusedX[2:5, :], in_=sv[:, :])

        # ---- persistent coefficient stash: [128, 2(C/S), 18(k), 64(j)] ----
        coeff = state.tile([128, 2, PBLK, NJ], F16)

        # ================= phase 1 unit: one (superchunk, m) =================
        def phase1_block(s, m, split_exp=False):
            js = SUPS[s]
            sup = js * 128
            b0 = SOFF[s] * 128
            e_in = epool.tile([NK, 3, sup], F16, name=f"ei_{s}_{m}", tag="ei")
            for k in range(3):
                nc.sync.dma_start(
                    out=e_in[:, k, :], in_=lg[m, k * NK:(k + 1) * NK, b0:b0 + sup]
                )
            e_t = xpool.tile([NK, 3, sup], F16, name=f"e_{s}_{m}", tag="e")
            if split_exp:
                for k in range(3):
                    nc.scalar.activation(out=e_t[:, k, :], in_=e_in[:, k, :],
                                         func=AF.Exp)
            else:
                nc.scalar.activation(out=e_t[:], in_=e_in[:], func=AF.Exp)
            ps = psum.tile([128, js, NCOL], F32, name=f"mom_{s}_{m}", tag="mom")
            for j in range(js):
                cols = j * 128
                for k in range(3):
                    nc.tensor.matmul(
                        ps[:, j, :], e_t[:, k, cols:cols + 128], fp_t[:, k, :],
                        start=(k == 0), stop=(k == 2),
                    )
            return ps

        # ---- accumulation closures (DVE: recip + merged stt; adds on the
        # engine that owns the consuming chain, to avoid cross-engine gating)
        def accum_closures(s, pss, add_eng):
            """closure list for superchunk s: 6 DVE ops + 2 add ops."""
            # flattened [(c h)] views: TensorScalarPtr-class ops allow only
            # 2 free dims, and the (c, h) axes merge to one stride cleanly
            js = SUPS[s]
            jc = SOFF[s]
            cslf = coeff.rearrange(
                "p c h j -> p (c h) j")[:, :, jc:jc + js]   # [128, 36, js]
            dve, adds = [], []
            for m in range(3):
                ps = pss[m]
                rz = rpool.tile([128, js], F32, name=f"rz_{s}_{m}", tag="rz")
                psCS = ps[:, :, 1:NCOL].transpose([0, 2, 1])  # [128, 36, js]
                rzb = rz[:, None, :].broadcast_to([128, 2 * PBLK, js])

                def c_rec(rz=rz, ps=ps):
                    nc.vector.reciprocal(rz[:], ps[:, :, 0])
                dve.append(c_rec)
                if m == 0:
                    def c_stt(psCS=psCS, rzb=rzb, cslf=cslf):
                        nc.vector.scalar_tensor_tensor(
                            cslf, psCS, 1.0 / 64.0, rzb, OP.mult, OP.mult)
                    dve.append(c_stt)
                else:
                    tm = rpool.tile([128, 2 * PBLK, js], F16,
                                    name=f"tm_{s}_{m}", tag="tm")

                    def c_stt(psCS=psCS, rzb=rzb, tm=tm):
                        nc.vector.scalar_tensor_tensor(
                            tm[:], psCS, 1.0 / 64.0, rzb, OP.mult, OP.mult)
                    dve.append(c_stt)

                    def c_add(tm=tm, cslf=cslf, e=add_eng):
                        e.tensor_tensor(cslf, cslf, tm[:], OP.add)
                    adds.append(c_add)
            return dve, adds

        # ================= phase 2: per-superchunk chains =================
        class Chain:
            def __init__(self, s, label, eng, w=None):
                self.jlo, self.w = SOFF[s], (w or SUPS[s])
                w = self.w
                self.eng = eng
                self.pool = eng is nc.gpsimd
                self.label = label
                w_ = w
                self.P = state.tile([128, PBLK, w_], F16, name=f"P_{label}")
                self.csl = coeff[:, :, :, self.jlo:self.jlo + w_]
                self.prod = state.tile([128, 2, PBLK, w_], F16, name=f"pr_{label}")
                self.CS = state.tile([128, 2, w_], F32, name=f"CS_{label}")
                self.sq2 = state.tile([128, 2, w_], F32, name=f"sq_{label}")
                self.r2 = state.tile([128, 1, w_], F32, name=f"r2_{label}")
                self.t2 = state.tile([128, 1, w_], F32, name=f"t2_{label}")
                self.yn = state.tile([128, 1, w_], F32, name=f"yn_{label}")
                self.pt1 = state.tile([128, 2, KORD // 2, w_], F16,
                                      name=f"p1_{label}")
                self.pt2 = state.tile([128, 2, KORD // 2, w_], F16,
                                      name=f"p2_{label}")
                self.q9 = state.tile([128, 2, 9, w_], F16, name=f"q9_{label}")
                self.q4 = state.tile([128, 2, 4, w_], F16, name=f"q4_{label}")
                self.q2 = state.tile([128, 2, 2, w_], F16, name=f"q2_{label}")
                self.q1 = state.tile([128, 2, w_], F16, name=f"q1_{label}")
                self.tpin = state.tile([128, 128], F16, name=f"tp_{label}")

            # gpsimd supports only the TensorTensor class of elementwise ops
            def tt(self, o, a, b, op):
                self.eng.tensor_tensor(o, a, b, op)

            def init(self):
                # init runs on DVE even for Pool chains (copy/scalar ops are
                # not available on gpsimd, and this also shortens the chain's
                # serial prologue)
                v = nc.vector
                v.memset(self.P[:, 0:1, :], 1.0)
                v.memset(self.P[:, NPC:PBLK, :], 0.0)
                v.tensor_copy(self.CS[:], self.csl[:, :, 0, :])
                self.normalize(lambda: None, eng=v)

            def _doubling(self, F):
                # two paired-product ops per stage (3 free dims max):
                # pt1 = (cm, sm) * cj, pt2 = (cm, sm) * sj; then 2 combines.
                P, w = self.P, self.w
                mlen = 1
                while mlen < KORD:
                    ww = min(mlen, KORD - mlen)
                    cmsm = P[:, CI + mlen - 1:SI + mlen:SI - CI, :]
                    cms = cmsm[:, :, None, :].broadcast_to([128, 2, ww, w])
                    cj = P[:, CI:CI + ww, :][:, None, :, :].broadcast_to(
                        [128, 2, ww, w])
                    sj = P[:, SI:SI + ww, :][:, None, :, :].broadcast_to(
                        [128, 2, ww, w])
                    t1 = self.pt1[:, :, 0:ww, :]
                    t2 = self.pt2[:, :, 0:ww, :]
                    self.tt(t1, cms, cj, OP.mult)   # (cm*cj, sm*cj)
                    self.tt(t2, cms, sj, OP.mult)   # (cm*sj, sm*sj)
                    self.tt(P[:, CI + mlen:CI + mlen + ww, :],
                            t1[:, 0], t2[:, 1], OP.subtract)
                    self.tt(P[:, SI + mlen:SI + mlen + ww, :],
                            t1[:, 1], t2[:, 0], OP.add)
                    mlen += ww
                    F()

            def _prod_reduce(self, F):
                Pb = self.P[:, None, :, :].broadcast_to([128, 2, PBLK, self.w])
                self.tt(self.prod[:], self.csl, Pb, OP.mult)
                F()
                # fp16 add-tree beats tensor_reduce on DVE (2x mode applies
                # to the packed adds but not to the reduce) and is the only
                # option on Pool (gpsimd cannot X-reduce). Level 1 pairs
                # harmonics via even/odd slices of the flattened (c h) axis.
                prf = self.prod.rearrange("p c h w -> p (c h) w")
                q9f = self.q9.rearrange("p c h w -> p (c h) w")
                self.tt(q9f, prf[:, 0:2 * PBLK:2, :],
                        prf[:, 1:2 * PBLK:2, :], OP.add)
                self.tt(self.q4[:], self.q9[:, :, 0:8:2, :],
                        self.q9[:, :, 1:8:2, :], OP.add)
                self.tt(self.q2[:], self.q4[:, :, 0:4:2, :],
                        self.q4[:, :, 1:4:2, :], OP.add)
                self.tt(self.q1[:], self.q2[:, :, 0, :],
                        self.q2[:, :, 1, :], OP.add)
                self.tt(self.CS[:], self.q1[:], self.q9[:, :, 8, :], OP.add)
                F()

            def normalize(self, F, zout=None, eng=None):
                e = eng or self.eng
                w = self.w
                r2, t2, yn = self.r2[:, 0, :], self.t2[:, 0, :], self.yn[:, 0, :]
                r2i = self.r2.bitcast(I32)[:, 0, :]
                yi = self.yn.bitcast(I32)[:, 0, :]
                zrows = zout if zout is not None \
                    else self.P[:, CI:SI + 1:SI - CI, :]
                ynb = self.yn.broadcast_to([128, 2, w])
                e.tensor_tensor(self.sq2[:], self.CS[:], self.CS[:], OP.mult)
                e.tensor_tensor(r2, self.sq2[:, 0, :], self.sq2[:, 1, :],
                                OP.add)
                F()
                if e is nc.gpsimd:
                    e.tensor_tensor(yi, r2i, ones_i[:, 0:w],
                                    OP.logical_shift_right)
                    e.tensor_tensor(yi, magic_i[:, 0:w], yi, OP.subtract)
                else:
                    e.tensor_scalar(yi, r2i, 1, None, OP.logical_shift_right)
                    e.tensor_scalar(yi, yi, -1, MAGIC, OP.mult, OP.add)
                F()
                if NEWTON:
                    e.tensor_tensor(t2, yn, yn, OP.mult)
                    e.tensor_tensor(t2, t2, r2, OP.mult)
                    if e is nc.gpsimd:
                        raise NotImplementedError
                    e.tensor_scalar(t2, t2, -0.5, 1.5, OP.mult, OP.add)
                    F()
                    e.tensor_tensor(yn, yn, t2, OP.mult)
                e.tensor_tensor(zrows, self.CS[:], ynb, OP.mult)
                F()

            def iter_once(self, fill=(), zout=None, norm_defer=None):
                fill = list(fill)

                def F():
                    if fill:
                        fill.pop(0)()
                self._doubling(F)
                self._prod_reduce(F)
                if norm_defer is None:
                    self.normalize(F, zout)
                else:
                    # Pool chains: gpsimd has no scalar/int ops, so the
                    # normalize is deferred to a closure the DVE stream runs
                    # at a matching point (paced fills in the host chains)
                    def nrm(z=zout):
                        self.normalize(lambda: None, z, eng=nc.vector)
                    norm_defer.append(nrm)
                while fill:
                    fill.pop(0)()

        # ================= head MLP (per chain, w=16) =================
        # tpin cols 0:w = cos, w:2w = sin (written by the chain's last
        # normalize); PE-transpose -> b-major -> fusedX rows 0,1 -> MLP.
        out_all = state.tile([128, NJ, 2], F32)
        sqh = state.tile([128, NJ, 2], F32)
        r2o = state.tile([128, NJ], F32)
        yo = state.tile([128, NJ], F32)
        to = state.tile([128, NJ], F32)

        def head_pre(cd, use_act, mid=None, dma_eng=None, mid2=None, mid2_at=None):
            # PE-transpose the final unit vectors to j-major, then DMA the
            # rows into the b-major fused input (baseline-proven pattern;
            # every head here runs after the exp stream drains, so PE/ACT
            # are free)
            jlo, w = cd.jlo, cd.w
            de = dma_eng or nc.sync
            pst = psumh.tile([128, 128], F16, name=f"pst_{jlo}", tag="h")
            nc.tensor.transpose(pst[:], cd.tpin[:], eye_t[:])
            csfT = headp.tile([128, 128], F16, name=f"csfT_{jlo}", tag="csfT")
            nc.scalar.activation(out=csfT[0:2 * w, :], in_=pst[0:2 * w, :],
                                 func=AF.Copy)
            de.dma_start(
                out=fusedX[0:2, jlo * 128:(jlo + w) * 128].rearrange(
                    "r (j p) -> r j p", p=128),
                in_=csfT[0:2 * w, :],
            )
            if mid is not None:
                mid()
            ps2 = psumh.tile([128, w, 2], F32, name=f"o_{jlo}", tag="o")
            cd.ps2 = ps2
            for jj in range(0, w, 4):
                j = jlo + jj
                ps1 = psumh.tile([128, 4, 128], F32, name=f"h_{j}", tag="h")
                for u in range(4):
                    nc.tensor.matmul(
                        ps1[:, u, :], w1b_t[:],
                        fusedX[0:W1R, (j + u) * 128:(j + u + 1) * 128],
                        start=True, stop=True,
                    )
                hT = headp.tile([128, 4, 128], F16, name=f"hT_{j}", tag="hT")
                if use_act or (jj // 4) % 2:
                    nc.scalar.activation(out=hT[:], in_=ps1[:], func=AF.Relu)
                else:
                    nc.vector.tensor_scalar(hT[:], ps1[:], 0.0, None, OP.max)
                for u in range(4):
                    nc.tensor.matmul(ps2[:, jj + u, :], hT[:, u, :], w2_t[:],
                                     start=True, stop=True)
                if mid2 is not None and jj == mid2_at:
                    mid2()
            return ps2

        def head_post(cd, ps2, emit=True, sub=None):
            jlo, w = cd.jlo, cd.w
            if sub is not None:
                jlo, w = jlo + sub[0], sub[1]
                ps2 = ps2[:, sub[0]:sub[0] + sub[1], :]
            oa = out_all[:, jlo:jlo + w, :]
            sh = sqh[:, jlo:jlo + w, :]
            r2h = r2o[:, jlo:jlo + w]
            yh = yo[:, jlo:jlo + w]
            th = to[:, jlo:jlo + w]
            r2i, yi = r2h.bitcast(I32), yh.bitcast(I32)
            v = nc.vector
            ops = [
                lambda: v.tensor_tensor(
                    oa, ps2[:, 0:w, :],
                    b2r_t[:, None, :].broadcast_to([128, w, 2]), OP.add),
                lambda: v.tensor_tensor(sh, oa, oa, OP.mult),
                lambda: v.tensor_tensor(r2h, sh[:, :, 0], sh[:, :, 1], OP.add),
                lambda: v.tensor_scalar(yi, r2i, 1, None,
                                        OP.logical_shift_right),
                lambda: v.tensor_scalar(yi, yi, -1, MAGIC, OP.mult, OP.add),
            ]
            for _ in range(1):
                ops.append(lambda: v.tensor_tensor(th, yh, yh, OP.mult))
                ops.append(lambda: v.tensor_tensor(th, th, r2h, OP.mult))
                ops.append(lambda: v.tensor_scalar(th, th, -0.5, 1.5,
                                                   OP.mult, OP.add))
                ops.append(lambda: v.tensor_tensor(yh, yh, th, OP.mult))
            ops.append(lambda: v.tensor_scalar(yh, yh, 1e12, None, OP.min))
            ops.append(lambda: v.tensor_tensor(
                oa, oa, yh[:, :, None].broadcast_to([128, w, 2]), OP.mult))
            # out stores ride the ACT queue: keeps the sync queue free for
            # the fused-row DMAs of later heads (FIFO head-of-line blocking)
            ops.append(lambda: nc.scalar.dma_start(
                out=out[jlo * 128:(jlo + w) * 128].rearrange(
                    "(j p) c -> p j c", p=128),
                in_=oa,
            ))
            if emit:
                for op in ops:
                    op()
                return []
            return ops

        # ================= schedule =================
        # phase 1 emission (fixes DMA/ACT/PE program order)
        pss = {}
        pss[(0, 0)] = phase1_block(0, 0, split_exp=True)
        pss[(0, 1)] = phase1_block(0, 1)
        pss[(0, 2)] = phase1_block(0, 2)
        load_late_consts()
        for s in range(1, NSUP):
            for m in range(3):
                pss[(s, m)] = phase1_block(s, m)

        def chain_iters(cd, fills_by_iter):
            zv = cd.tpin[:, 0:2 * cd.w].rearrange("p (r w) -> p r w", r=2)
            nc.vector.memset(cd.tpin[:, 2 * cd.w:128], 0.0)
            for it in range(MS_ITERS):
                cd.iter_once(fill=fills_by_iter.get(it, ()),
                             zout=zv if it == MS_ITERS - 1 else None)

        # ---- DVE: s0 accumulate (adds on DVE: gates A1), then chain A1 ----
        acc0, add0 = accum_closures(0, [pss[(0, m)] for m in range(3)],
                                    nc.vector)
        for c in acc0 + add0:
            c()
        A1 = Chain(0, "A1", nc.vector)
        A1.init()
        # s1 accum rides A1 iters, timed to the exp stream (psum s1 ready
        # ~[25, 30, 36]; A1 iter k starts ~20+2.6k); s1 adds go to Pool
        # (they gate only the Pool chain B).
        acc1, add1 = accum_closures(1, [pss[(1, m)] for m in range(3)],
                                    nc.vector)
        acc2, add2 = accum_closures(2, [pss[(2, m)] for m in range(3)],
                                    nc.vector)
        chain_iters(A1, {3: acc1[0:2], 6: acc1[2:4], 9: acc1[4:6]})
        # s1 chain on DVE right after s0's (gpsimd lacks the scalar/int op
        # classes a chain needs, and cross-engine chain deps resolve against
        # the tile scheduler's optimistic Pool timing — chains stay on DVE)
        acc3, add3 = accum_closures(3, [pss[(3, m)] for m in range(3)],
                                    nc.vector)
        Bc = Chain(1, "B", nc.vector)
        for c in add1:
            c()
        Bc.init()
        chain_iters(Bc, {0: acc2[0:2], 2: acc2[2:4], 4: acc2[4:6] + add2,
                         6: acc3[0:2], 8: acc3[2:4], 9: acc3[4:6] + add3})

        # ---- heads emitted in execution order (PE/ACT are in-order!) ----
        ps2_0 = head_pre(A1, use_act=True)    # runs right after exp drains
        post0 = head_post(A1, ps2_0, emit=False)
        ps2_1 = head_pre(Bc, use_act=True)
        post1 = head_post(Bc, ps2_1, emit=False)

        # ---- DVE: merged tail chain C (s2+s3, w=32): by the time B ends,
        # both coefficient sets are ready, and one w32 chain pays the
        # per-op overhead once instead of twice ----
        C = Chain(2, "C", nc.vector, w=SUPS[2] + SUPS[3])
        C.init()
        chain_iters(C, {2: post0[0:5], 3: post0[5:10], 4: post0[10:16],
                        5: post1[0:5], 6: post1[5:10], 7: post1[10:16]})
        HW2 = (SUPS[2] + SUPS[3]) // 2
        ps2_2 = head_pre(
            C, use_act=False, mid2_at=4 * ((HW2 + 3) // 4) - 4,
            mid2=lambda: head_post(C, C.ps2, emit=True, sub=(0, HW2)))
        head_post(C, ps2_2, emit=True, sub=(HW2, SUPS[2] + SUPS[3] - HW2))


def _build_Fp():
    """F' [3, NK, NCOL] fp16: exp-logits -> [Z | coeffC+pad | coeffS+pad]."""
    iv10 = [
        2815.716628466254, 2670.988303701255, 2281.518967726004,
        1758.380716166120, 1226.490565693291, 777.1882064830589,
        449.3022898718774, 238.0255847757819, 116.0661461102767,
        52.31922632375539, 21.89170616206518, 8.536924495442690,
        3.119276255343020, 1.071597692949700,
    ]
    K = KORD
    n = np.arange(N)
    th = 2 * np.pi * n / N
    c = np.array([iv10[0]] + [2 * iv10[k] for k in range(1, K + 2)])
    A = np.cos(np.outer(np.arange(K + 2), th))   # [K+2, N]
    Bm = np.sin(np.outer(np.arange(K + 2), th))
    cols = [np.ones(N)]
    cols.append(c[0] * A[1])                          # CA_0
    for k in range(1, K + 1):
        cols.append(c[k] / 2 * (A[k - 1] + A[k + 1]))  # CA_k
    for k in range(1, K + 1):
        cols.append(c[k] / 2 * (Bm[k - 1] + Bm[k + 1]))  # CB_k
    cols.append(np.zeros(N))                          # pad
    cols.append(c[0] * Bm[1])                         # SB_0
    for k in range(1, K + 1):
        cols.append(c[k] / 2 * (Bm[k + 1] - Bm[k - 1]))  # SB_k
    for k in range(1, K + 1):
        cols.append(c[k] / 2 * (A[k - 1] - A[k + 1]))  # SA_k
    cols.append(np.zeros(N))                          # pad
    Fp = np.stack(cols, axis=1).astype(np.float16)    # [N, NCOL]
    assert Fp.shape[1] == NCOL
    return np.ascontiguousarray(Fp.reshape(3, NK, NCOL))


_NC_CACHE = {}


def _get_nc():
    if "nc" not in _NC_CACHE:
        nc = bacc.Bacc("TRN2", target_bir_lowering=False, debug=False,
                       enable_asserts=True, num_devices=NCORES)
        build(nc)
        nc.compile()
        _NC_CACHE["nc"] = nc
    return _NC_CACHE["nc"]


def kernel(von_logits, sin_vec, W1, b1, W2, b2, _trace=False, _trace_kwargs=None):
    vT = np.ascontiguousarray(
        np.asarray(von_logits, np.float32).transpose(0, 2, 1).astype(np.float16)
    )  # [M, N, B] fp16
    svT = np.concatenate([
        np.asarray(sin_vec, np.float32).T,
        np.ones((1, B), np.float32),
    ], axis=0).astype(np.float16)  # [3, B] rows: sv0, sv1, ones
    Fp = _build_Fp()
    W1f = np.asarray(W1, np.float32)
    # fused input rows: [cos, sin, sv0, sv1, ones]
    W1b = np.ascontiguousarray(np.concatenate(
        [W1f[2:4], W1f[0:2], np.asarray(b1, np.float32)[None, :]], 0
    ).astype(np.float16))
    W2f = np.ascontiguousarray(np.asarray(W2, np.float32).astype(np.float16))
    b2rep = np.ascontiguousarray(np.broadcast_to(np.asarray(b2, np.float32), (128, 2)))
    eye16 = np.ascontiguousarray(np.eye(128, dtype=np.float16))

    in_maps = []
    for ci in range(NCORES):
        sl = slice(ci * BS, (ci + 1) * BS)
        in_maps.append({
            "logitsT": np.ascontiguousarray(vT[:, :, sl]),
            "sin_vecT": np.ascontiguousarray(svT[:, sl]),
            "Fp": Fp, "W1b": W1b, "W2": W2f, "b2r": b2rep, "eye": eye16,
        })

    nc = _get_nc()
    kw = {}
    if _trace:
        kw = {"trace": True, "trace_kwargs": _trace_kwargs or {}}
    res = run_bass_kernel_spmd(nc, in_maps, core_ids=list(range(NCORES)), **kw)
    outs = [r["out"] for r in res.results]
    full = np.concatenate(outs, axis=0).astype(np.float32)
    if _trace:
        kernel._last_results = res
    return full
